# revision 10
# baseline (speedup 1.0000x reference)
"""Bounding-box discipline penalty kernel for Trainium2 (8 NeuronCores).

Reference computation:
    pred_mask = max_c(prediction_probs) > 0.3   [B, H, W]
    true_mask = max_c(expected_onehot)  > 0.5   [B, H, W]
    per-sample bboxes from the masks -> area/center penalties -> scalar mean.

Strategy (pure data parallel, B=16 over 8 cores => 2 samples/core):
  * Device: stream both tensors' shards through SBUF and compute the
    per-pixel channel max with the Vector engine. The job is one
    128 MiB/core HBM read that saturates all 16 DMA engines at line rate
    (~27 GB/s each); the kernel keeps that stream bubble-free and
    minimizes the head/tail overhead around it.
  * Schedule: 8 MiB chunks (64 KiB descriptors, best per-descriptor
    efficiency) for the bulk, tapering to 2 MiB / 1 MiB chunks at the end
    so the final reduce drain after the last DMA packet is ~4.4 us
    instead of ~19 us.
  * Bulk-chunk reduces are gated on the completion of the FOLLOWING load
    (one-chunk lookahead): the DMA completion semaphore does not
    guarantee every DMA engine's FIFO has drained, and profiler-induced
    engine stalls can otherwise expose reads of partially-written
    regions. The lookahead buys ~10 us of settle margin at zero cost in
    the bulk; the final tapered chunks use their own semaphores to keep
    the drain short.
  * Host: fold the tiny [4, 128, 512] per-core pixel-max results into
    per-sample row/col maxima (exact max ops), then do the O(B) bbox +
    penalty math exactly as the reference does.

Self-contained: hardcodes shapes from the problem spec.
"""

import numpy as np

THRESHOLD = 0.3
PENALTY_WEIGHT = 0.05

B, H, W, C = 16, 256, 256, 128
N_CORES = 8
SPC = B // N_CORES            # samples per core = 2
NST = 2 * SPC                 # sample-tensor streams per core = 4
PIX = H * W                   # 65536 pixels per sample
NPART = 128
PPP = PIX // NPART            # 512 pixels per partition
EPP = PPP * C                 # 65536 f32 elems per partition per sample
CIRC = 49152                  # circular SBUF buffer (elems/partition)
NLS = 8                       # recycled load-semaphore pool size

# chunk sizes (elems/partition): 64 KiB-descriptor bulk, then a plateau
# taper (8192s then 4096s) sized so the vector engine's reduce backlog is
# nearly zero when the last packet lands
SIZES = [16384] * 8 + [8192] * 11 + [4096] * 10
assert sum(SIZES) == NST * EPP
# chunks below this index gate their reduce on the NEXT load's semaphore
# (settle margin against DMA-engine FIFO skew at kernel start, where a
# stale read could see zeroed SBUF and flip a mask bit); deeper lookahead
# would stall the 3-deep slot ring on 16384-elem chunks
LOOKAHEAD_END = 3

_cache = {}


def _plan():
    """(st, stream elem offset, size, circular offset) per load."""
    plan = []
    cum = 0
    for sz in SIZES:
        st, off = divmod(cum, EPP)
        c = cum % CIRC
        assert c + sz <= CIRC, "chunk crosses circular wrap"
        plan.append((st, off, sz, c))
        cum += sz
    return plan


def _build_nc():
    from contextlib import ExitStack

    import concourse.bass as bass
    import concourse.mybir as mybir

    f32 = mybir.dt.float32
    nc = bass.Bass()
    pred = nc.dram_tensor("pred", [SPC, NPART, EPP], f32, kind="ExternalInput")
    tru = nc.dram_tensor("tru", [SPC, NPART, EPP], f32, kind="ExternalInput")
    # pixmax per sample-tensor: [st, partition, pixel-in-partition]
    outp = nc.dram_tensor("outp", [NST, NPART, PPP], f32, kind="ExternalOutput")

    srcs = [(pred, 0), (pred, 1), (tru, 0), (tru, 1)]
    plan = _plan()
    nloads = len(plan)

    # gate[k]: latest earlier load whose circular region overlaps load k's
    # (its reduce must complete before load k may overwrite), or -1
    gate = [-1] * nloads
    for k, (_st, _off, sz, c) in enumerate(plan):
        for j in range(k - 1, -1, -1):
            _stj, _offj, szj, cj = plan[j]
            if cj < c + sz and c < cj + szj:
                gate[k] = j
                break

    # last reduce index (1-based count) needed before flushing each st
    last_of_st = {}
    for k, (st, _o, _s, _c) in enumerate(plan):
        last_of_st[st] = k + 1

    SLOT = 16384
    with ExitStack() as ctx:
        slots = [
            ctx.enter_context(nc.sbuf_tensor(f"buf{i}", [NPART, SLOT], f32))
            for i in range(CIRC // SLOT)
        ]

        def bufsl(c, sz):
            sl, soff = divmod(c, SLOT)
            assert soff + sz <= SLOT
            return slots[sl][:, soff : soff + sz]

        pm = [
            ctx.enter_context(nc.sbuf_tensor(f"pm{i}", [NPART, PPP], f32))
            for i in range(NST)
        ]
        lsems = [
            ctx.enter_context(nc.semaphore(f"ls{i}")) for i in range(NLS)
        ]
        vfree = ctx.enter_context(nc.semaphore("vfree"))
        outsem = ctx.enter_context(nc.semaphore("outsem"))
        block = ctx.enter_context(nc.Block())

        @block.sync
        def _(sync):
            for k, (st, off, sz, c) in enumerate(plan):
                src, s = srcs[st]
                if gate[k] >= 0:
                    sync.wait_ge(vfree, gate[k] + 1)
                sync.dma_start(
                    out=bufsl(c, sz),
                    in_=src[s, :, off : off + sz],
                ).then_inc(lsems[k % NLS], 16)

        @block.vector
        def _(vector):
            for k, (st, off, sz, c) in enumerate(plan):
                # head: wait for the NEXT load too (engine-FIFO settle);
                # rest: wait only own load to keep the pipeline tight
                g = k + 1 if k + 1 < LOOKAHEAD_END else k
                vector.wait_ge(lsems[g % NLS], 16 * (g // NLS + 1))
                if g != k:
                    vector.wait_ge(lsems[k % NLS], 16 * (k // NLS + 1))
                vector.reduce_max(
                    out=pm[st][:, off // C : (off + sz) // C],
                    in_=bufsl(c, sz).rearrange("p (a c) -> p a c", c=C),
                    axis=mybir.AxisListType.X,
                ).then_inc(vfree, 1)

        @block.scalar
        def _(scalar):
            def flush(st, px_lo, px_hi, need_v):
                scalar.wait_ge(vfree, need_v)
                scalar.dma_start(
                    out=outp[st, :, px_lo:px_hi],
                    in_=pm[st][:, px_lo:px_hi],
                ).then_inc(outsem, 16)

            for st in range(NST - 1):
                flush(st, 0, PPP, last_of_st[st])
            # final stream: flush all but the last chunk's pixels early so
            # only a sliver trails the final reduce; the framework's
            # end-of-engine DRAIN waits for the queue, no explicit wait
            last_px = SIZES[-1] // C
            flush(NST - 1, 0, PPP - last_px, nloads - 1)
            flush(NST - 1, PPP - last_px, PPP, nloads)

    return nc


def _run_device(pred_np, true_np, trace=False):
    from concourse.bass_utils import run_bass_kernel_spmd

    if "nc" not in _cache:
        _cache["nc"] = _build_nc()
    nc = _cache["nc"]

    # [B, H, W, C] -> per-core shards [SPC, 128, EPP]
    pred_sh = pred_np.reshape(N_CORES, SPC, NPART, EPP)
    true_sh = true_np.reshape(N_CORES, SPC, NPART, EPP)
    in_maps = [
        {"pred": pred_sh[i], "tru": true_sh[i]} for i in range(N_CORES)
    ]
    res = run_bass_kernel_spmd(
        nc, in_maps, core_ids=list(range(N_CORES)), trace=trace
    )
    # [N_CORES, NST, 128, PPP]
    pms = np.stack([res.results[i]["outp"] for i in range(N_CORES)])
    return pms, res


def _bbox_from_maxes(rowv, colv, thresh):
    """rowv [B,H], colv [B,W] float32 maxima -> bbox coords, matching _bbox."""
    row_any = rowv > thresh
    col_any = colv > thresh
    ys = np.arange(H, dtype=np.float32)
    xs = np.arange(W, dtype=np.float32)
    y_min = np.where(row_any, ys, np.float32(H)).min(axis=1)
    y_max = np.where(row_any, ys, np.float32(-1)).max(axis=1)
    x_min = np.where(col_any, xs, np.float32(W)).min(axis=1)
    x_max = np.where(col_any, xs, np.float32(-1)).max(axis=1)
    empty = ~row_any.any(axis=1)
    f32 = np.float32
    y_min = np.where(empty, f32(0.0), y_min).astype(np.float32)
    x_min = np.where(empty, f32(0.0), x_min).astype(np.float32)
    y_max = np.where(empty, f32(1.0), y_max).astype(np.float32)
    x_max = np.where(empty, f32(1.0), x_max).astype(np.float32)
    return y_min, x_min, y_max, x_max


def _fold_pms(pms):
    """pms [N_CORES, NST, 128, PPP] -> rowv, colv each [2, B, 256].

    pms[c, st] covers sample 2c + (st % SPC) of tensor st // SPC; partition
    p holds image rows 2p, 2p+1 as two 256-pixel runs.
    """
    pm4 = pms.reshape(N_CORES, 2, SPC, NPART, 2, W)  # [c, tensor, s, p, r, w]
    pm4 = pm4.transpose(1, 0, 2, 3, 4, 5).reshape(2, B, NPART, 2, W)
    rowv = pm4.max(axis=4).reshape(2, B, H)   # rows h = 2p + r
    colv = pm4.max(axis=(2, 3))               # [2, B, W]
    return rowv, colv


def _penalty_from_pms(pms):
    """pms [N_CORES, NST, 128, PPP] -> scalar penalty (float32)."""
    rowv, colv = _fold_pms(pms)
    p = _bbox_from_maxes(rowv[0], colv[0], np.float32(THRESHOLD))
    t = _bbox_from_maxes(rowv[1], colv[1], np.float32(0.5))
    py_min, px_min, py_max, px_max = p
    ty_min, tx_min, ty_max, tx_max = t

    one = np.float32(1.0)
    pred_area = (py_max - py_min + one) * (px_max - px_min + one)
    true_area = (ty_max - ty_min + one) * (tx_max - tx_min + one)
    area_penalty = np.maximum(pred_area - true_area, np.float32(0.0)) / (
        true_area + one
    )
    two = np.float32(2.0)
    dy = (py_min + py_max) / two - (ty_min + ty_max) / two
    dx = (px_min + px_max) / two - (tx_min + tx_max) / two
    center_offset = np.sqrt(dy * dy + dx * dx).astype(np.float32) / np.float32(
        20.0
    )
    penalties = area_penalty + center_offset
    return np.float32(PENALTY_WEIGHT) * penalties.mean(dtype=np.float32)


def _run(prediction_probs, expected_onehot, trace=False):
    pred_np = np.ascontiguousarray(
        np.asarray(prediction_probs, dtype=np.float32)
    )
    true_np = np.ascontiguousarray(
        np.asarray(expected_onehot, dtype=np.float32)
    )
    assert pred_np.shape == (B, H, W, C), pred_np.shape
    assert true_np.shape == (B, H, W, C), true_np.shape
    pms, res = _run_device(pred_np, true_np, trace=trace)
    val = _penalty_from_pms(pms)
    return np.asarray(val, dtype=np.float32), res


def kernel(prediction_probs, expected_onehot):
    out, _ = _run(prediction_probs, expected_onehot, trace=False)
    return out


# revision 11
# speedup vs baseline: 1.2129x; 1.2129x over previous
"""Bounding-box discipline penalty kernel for Trainium2 (8 NeuronCores).

Reference computation:
    pred_mask = max_c(prediction_probs) > 0.3   [B, H, W]
    true_mask = max_c(expected_onehot)  > 0.5   [B, H, W]
    per-sample bboxes from the masks -> area/center penalties -> scalar mean.

Strategy (pure data parallel, B=16 over 8 cores => 2 samples/core):
  * Device: stream both tensors' shards through SBUF and compute the
    per-pixel channel max with the Vector engine. The job is one
    128 MiB/core HBM read that saturates all 16 DMA engines at line rate
    (~27 GB/s each); the kernel keeps that stream bubble-free and
    minimizes the head/tail overhead around it.
  * Schedule: 8 MiB chunks (64 KiB descriptors, best per-descriptor
    efficiency) for the bulk, tapering to 2 MiB / 1 MiB chunks at the end
    so the final reduce drain after the last DMA packet is ~4.4 us
    instead of ~19 us.
  * Bulk-chunk reduces are gated on the completion of the FOLLOWING load
    (one-chunk lookahead): the DMA completion semaphore does not
    guarantee every DMA engine's FIFO has drained, and profiler-induced
    engine stalls can otherwise expose reads of partially-written
    regions. The lookahead buys ~10 us of settle margin at zero cost in
    the bulk; the final tapered chunks use their own semaphores to keep
    the drain short.
  * Host: fold the tiny [4, 128, 512] per-core pixel-max results into
    per-sample row/col maxima (exact max ops), then do the O(B) bbox +
    penalty math exactly as the reference does.

Self-contained: hardcodes shapes from the problem spec.
"""

import numpy as np

THRESHOLD = 0.3
PENALTY_WEIGHT = 0.05

B, H, W, C = 16, 256, 256, 128
N_CORES = 8
SPC = B // N_CORES            # samples per core = 2
NST = 2 * SPC                 # sample-tensor streams per core = 4
PIX = H * W                   # 65536 pixels per sample
NPART = 128
PPP = PIX // NPART            # 512 pixels per partition
EPP = PPP * C                 # 65536 f32 elems per partition per sample
CIRC = 49152                  # circular SBUF buffer (elems/partition)
NLS = 8                       # recycled load-semaphore pool size

# chunk sizes (elems/partition): 64 KiB-descriptor bulk, then a plateau
# taper (8192s then 4096s) sized so the vector engine's reduce backlog is
# nearly zero when the last packet lands
SIZES = [16384] * 8 + [8192] * 11 + [4096] * 10
assert sum(SIZES) == NST * EPP
# Chunk 0 gates its reduce on load 1's semaphore: settle margin against
# DMA-engine FIFO skew at kernel start, where a stale read could see
# zeroed SBUF and flip a mask bit. Chunk 1+ are settled transitively by
# chunk 0's ~17 us reduce. Deeper lookahead is HARMFUL: once the vector
# engine's backlog nears ring-1 chunks, every slot-ring gate becomes
# binding and the whole pipeline locks to reduce-pace + semaphore/issue
# latency (~20% slower, measured 405 us vs 334 us).
LOOKAHEAD_END = 2

_cache = {}


def _plan():
    """(st, stream elem offset, size, circular offset) per load."""
    plan = []
    cum = 0
    for sz in SIZES:
        st, off = divmod(cum, EPP)
        c = cum % CIRC
        assert c + sz <= CIRC, "chunk crosses circular wrap"
        plan.append((st, off, sz, c))
        cum += sz
    return plan


def _build_nc():
    from contextlib import ExitStack

    import concourse.bass as bass
    import concourse.mybir as mybir

    f32 = mybir.dt.float32
    nc = bass.Bass()
    pred = nc.dram_tensor("pred", [SPC, NPART, EPP], f32, kind="ExternalInput")
    tru = nc.dram_tensor("tru", [SPC, NPART, EPP], f32, kind="ExternalInput")
    # pixmax per sample-tensor: [st, partition, pixel-in-partition]
    outp = nc.dram_tensor("outp", [NST, NPART, PPP], f32, kind="ExternalOutput")

    srcs = [(pred, 0), (pred, 1), (tru, 0), (tru, 1)]
    plan = _plan()
    nloads = len(plan)

    # gate[k]: latest earlier load whose circular region overlaps load k's
    # (its reduce must complete before load k may overwrite), or -1
    gate = [-1] * nloads
    for k, (_st, _off, sz, c) in enumerate(plan):
        for j in range(k - 1, -1, -1):
            _stj, _offj, szj, cj = plan[j]
            if cj < c + sz and c < cj + szj:
                gate[k] = j
                break

    # last reduce index (1-based count) needed before flushing each st
    last_of_st = {}
    for k, (st, _o, _s, _c) in enumerate(plan):
        last_of_st[st] = k + 1

    SLOT = 16384
    with ExitStack() as ctx:
        slots = [
            ctx.enter_context(nc.sbuf_tensor(f"buf{i}", [NPART, SLOT], f32))
            for i in range(CIRC // SLOT)
        ]

        def bufsl(c, sz):
            sl, soff = divmod(c, SLOT)
            assert soff + sz <= SLOT
            return slots[sl][:, soff : soff + sz]

        pm = [
            ctx.enter_context(nc.sbuf_tensor(f"pm{i}", [NPART, PPP], f32))
            for i in range(NST)
        ]
        lsems = [
            ctx.enter_context(nc.semaphore(f"ls{i}")) for i in range(NLS)
        ]
        vfree = ctx.enter_context(nc.semaphore("vfree"))
        outsem = ctx.enter_context(nc.semaphore("outsem"))
        block = ctx.enter_context(nc.Block())

        @block.sync
        def _(sync):
            for k, (st, off, sz, c) in enumerate(plan):
                src, s = srcs[st]
                if gate[k] >= 0:
                    sync.wait_ge(vfree, gate[k] + 1)
                sync.dma_start(
                    out=bufsl(c, sz),
                    in_=src[s, :, off : off + sz],
                ).then_inc(lsems[k % NLS], 16)

        @block.vector
        def _(vector):
            for k, (st, off, sz, c) in enumerate(plan):
                # head: wait for the NEXT load too (engine-FIFO settle);
                # rest: wait only own load to keep the pipeline tight
                g = k + 1 if k + 1 < LOOKAHEAD_END else k
                vector.wait_ge(lsems[g % NLS], 16 * (g // NLS + 1))
                if g != k:
                    vector.wait_ge(lsems[k % NLS], 16 * (k // NLS + 1))
                vector.reduce_max(
                    out=pm[st][:, off // C : (off + sz) // C],
                    in_=bufsl(c, sz).rearrange("p (a c) -> p a c", c=C),
                    axis=mybir.AxisListType.X,
                ).then_inc(vfree, 1)

        @block.scalar
        def _(scalar):
            def flush(st, px_lo, px_hi, need_v):
                scalar.wait_ge(vfree, need_v)
                scalar.dma_start(
                    out=outp[st, :, px_lo:px_hi],
                    in_=pm[st][:, px_lo:px_hi],
                ).then_inc(outsem, 16)

            for st in range(NST - 1):
                flush(st, 0, PPP, last_of_st[st])
            # final stream: flush all but the last chunk's pixels early so
            # only a sliver trails the final reduce; the framework's
            # end-of-engine DRAIN waits for the queue, no explicit wait
            last_px = SIZES[-1] // C
            flush(NST - 1, 0, PPP - last_px, nloads - 1)
            flush(NST - 1, PPP - last_px, PPP, nloads)

    return nc


def _run_device(pred_np, true_np, trace=False):
    from concourse.bass_utils import run_bass_kernel_spmd

    if "nc" not in _cache:
        _cache["nc"] = _build_nc()
    nc = _cache["nc"]

    # [B, H, W, C] -> per-core shards [SPC, 128, EPP]
    pred_sh = pred_np.reshape(N_CORES, SPC, NPART, EPP)
    true_sh = true_np.reshape(N_CORES, SPC, NPART, EPP)
    in_maps = [
        {"pred": pred_sh[i], "tru": true_sh[i]} for i in range(N_CORES)
    ]
    res = run_bass_kernel_spmd(
        nc, in_maps, core_ids=list(range(N_CORES)), trace=trace
    )
    # [N_CORES, NST, 128, PPP]
    pms = np.stack([res.results[i]["outp"] for i in range(N_CORES)])
    return pms, res


def _bbox_from_maxes(rowv, colv, thresh):
    """rowv [B,H], colv [B,W] float32 maxima -> bbox coords, matching _bbox."""
    row_any = rowv > thresh
    col_any = colv > thresh
    ys = np.arange(H, dtype=np.float32)
    xs = np.arange(W, dtype=np.float32)
    y_min = np.where(row_any, ys, np.float32(H)).min(axis=1)
    y_max = np.where(row_any, ys, np.float32(-1)).max(axis=1)
    x_min = np.where(col_any, xs, np.float32(W)).min(axis=1)
    x_max = np.where(col_any, xs, np.float32(-1)).max(axis=1)
    empty = ~row_any.any(axis=1)
    f32 = np.float32
    y_min = np.where(empty, f32(0.0), y_min).astype(np.float32)
    x_min = np.where(empty, f32(0.0), x_min).astype(np.float32)
    y_max = np.where(empty, f32(1.0), y_max).astype(np.float32)
    x_max = np.where(empty, f32(1.0), x_max).astype(np.float32)
    return y_min, x_min, y_max, x_max


def _fold_pms(pms):
    """pms [N_CORES, NST, 128, PPP] -> rowv, colv each [2, B, 256].

    pms[c, st] covers sample 2c + (st % SPC) of tensor st // SPC; partition
    p holds image rows 2p, 2p+1 as two 256-pixel runs.
    """
    pm4 = pms.reshape(N_CORES, 2, SPC, NPART, 2, W)  # [c, tensor, s, p, r, w]
    pm4 = pm4.transpose(1, 0, 2, 3, 4, 5).reshape(2, B, NPART, 2, W)
    rowv = pm4.max(axis=4).reshape(2, B, H)   # rows h = 2p + r
    colv = pm4.max(axis=(2, 3))               # [2, B, W]
    return rowv, colv


def _penalty_from_pms(pms):
    """pms [N_CORES, NST, 128, PPP] -> scalar penalty (float32)."""
    rowv, colv = _fold_pms(pms)
    p = _bbox_from_maxes(rowv[0], colv[0], np.float32(THRESHOLD))
    t = _bbox_from_maxes(rowv[1], colv[1], np.float32(0.5))
    py_min, px_min, py_max, px_max = p
    ty_min, tx_min, ty_max, tx_max = t

    one = np.float32(1.0)
    pred_area = (py_max - py_min + one) * (px_max - px_min + one)
    true_area = (ty_max - ty_min + one) * (tx_max - tx_min + one)
    area_penalty = np.maximum(pred_area - true_area, np.float32(0.0)) / (
        true_area + one
    )
    two = np.float32(2.0)
    dy = (py_min + py_max) / two - (ty_min + ty_max) / two
    dx = (px_min + px_max) / two - (tx_min + tx_max) / two
    center_offset = np.sqrt(dy * dy + dx * dx).astype(np.float32) / np.float32(
        20.0
    )
    penalties = area_penalty + center_offset
    return np.float32(PENALTY_WEIGHT) * penalties.mean(dtype=np.float32)


def _run(prediction_probs, expected_onehot, trace=False):
    pred_np = np.ascontiguousarray(
        np.asarray(prediction_probs, dtype=np.float32)
    )
    true_np = np.ascontiguousarray(
        np.asarray(expected_onehot, dtype=np.float32)
    )
    assert pred_np.shape == (B, H, W, C), pred_np.shape
    assert true_np.shape == (B, H, W, C), true_np.shape
    pms, res = _run_device(pred_np, true_np, trace=trace)
    val = _penalty_from_pms(pms)
    return np.asarray(val, dtype=np.float32), res


def kernel(prediction_probs, expected_onehot):
    out, _ = _run(prediction_probs, expected_onehot, trace=False)
    return out
